# revision 1
# baseline (speedup 1.0000x reference)
"""Trainium2 Bass kernel for LANLayer-style GNN message passing.

Reference computation (N=8192, DIM=256, HID=128, K=10):
    h = relu(x @ W_proj + b); hn = h / ||h||
    sim = (hn @ hn.T + 1)/2; probs = softmax(sim/T); topi = top_k(probs, 10)
    A_hat = one_hot(topi) minus diag plus eye; deg = 10 per row
    out = relu((A_hat/10) @ (h @ W_conv))

Key algebraic reductions used here:
  - softmax is rank-preserving per row -> top-k of probs == top-k of
    Z[i,j] = h_i . hn_j  (query unnormalized, key normalized).
  - The diagonal is always top-1 (Z[i,i] = ||h_i||), and the reference's
    "scatter, zero diag, add eye" keeps exactly the top-10 set, so every
    row degree is 10 and adj_norm = A_hat / 10.
  - Row-wise 10th/11th largest found exactly via per-512-group max8 +
    mini top-16 on the group winners; mask = (Z >= (v10+v11)/2).
  - out rows = relu(0.1 * mask @ support), computed as a dense bf16
    matmul with PE-transposed mask blocks (mask is exact 0/1 in bf16).

Sharding: 8 cores; all inputs replicated except each core also gets its
own 1024-row slice of x ("xloc") so one SPMD program computes rows
[c*1024, (c+1)*1024) without dynamic addressing.
"""

import numpy as np

import concourse.bass as bass
import concourse.mybir as mybir
import concourse.tile as tile
from concourse import bacc
from concourse.bass_utils import run_bass_kernel_spmd
from concourse.masks import make_identity

N, DIM, HID = 8192, 256, 128
NCORES = 8
NLOC = N // NCORES          # 1024 rows per core
CH = 512                    # node chunk for phase 0 / Z matmul rhs
NCH = N // CH               # 16
RT = NLOC // 128            # 8 row-tiles per core
JT = N // 128               # 64 j-tiles
GROUP = 2                   # row-tiles aggregated together in phase 2
F32 = mybir.dt.float32
BF16 = mybir.dt.bfloat16
NEG = -1.0e30


def _transpose_x_chunk(nc, pool, psum_tp, ident, dram_x, base_row, xT_sb):
    """Load 512 rows of x and produce xT chunk [256(2x128 part), 512] in SBUF."""
    xrows = []
    for r in range(4):
        t = pool.tile([128, DIM], F32, tag="xrow")
        nc.sync.dma_start(t[:], dram_x[base_row + r * 128 : base_row + (r + 1) * 128, :])
        xrows.append(t)
    for fb in range(2):
        pt = psum_tp.tile([128, 512], F32, tag="xtp")
        for r in range(4):
            nc.tensor.transpose(
                pt[:, r * 128 : (r + 1) * 128],
                xrows[r][:, fb * 128 : (fb + 1) * 128],
                ident[:],
            )
        nc.scalar.copy(xT_sb[fb][:], pt[:])


def build_nc():
    nc = bacc.Bacc(None, target_bir_lowering=False)

    x_t = nc.dram_tensor("x", [N, DIM], F32, kind="ExternalInput")
    xloc_t = nc.dram_tensor("xloc", [NLOC, DIM], F32, kind="ExternalInput")
    wp_t = nc.dram_tensor("W_proj", [DIM, HID], F32, kind="ExternalInput")
    bp_t = nc.dram_tensor("b_proj", [HID], F32, kind="ExternalInput")
    wc_t = nc.dram_tensor("W_conv", [HID, HID], F32, kind="ExternalInput")
    out_t = nc.dram_tensor("out", [NLOC, HID], F32, kind="ExternalOutput")

    with tile.TileContext(nc) as tc:
        with (
            tc.tile_pool(name="const", bufs=1) as cpool,
            tc.tile_pool(name="big", bufs=1) as big,
        ):
            # --- constants ---
            ident = cpool.tile([128, 128], F32)
            make_identity(nc, ident[:])
            identb = cpool.tile([128, 128], BF16)
            make_identity(nc, identb[:])
            ones_col = cpool.tile([128, 1], F32)
            nc.vector.memset(ones_col[:], 1.0)
            ones_row = cpool.tile([1, 128], F32)
            nc.vector.memset(ones_row[:], 1.0)
            wp_sb = cpool.tile([128, 2, HID], F32)
            nc.sync.dma_start(wp_sb[:], wp_t[:].rearrange("(k p) h -> p k h", p=128))
            b_sb = cpool.tile([128, 1], F32)
            nc.sync.dma_start(b_sb[:], bp_t[:].rearrange("(p one) -> p one", one=1))
            wc_sb = cpool.tile([128, HID], F32)
            nc.sync.dma_start(wc_sb[:], wc_t[:])

            # --- persistent big tensors ---
            hnT = big.tile([128, N], F32)          # normalized keys, [hid, j]
            hTloc = big.tile([128, NLOC], F32)     # unnormalized queries, [hid, i]
            supp = big.tile([128, JT, HID], BF16)  # support rows, [j%128, jt, hid]

            # ---------------- phase 0 ----------------
            with (
                tc.tile_pool(name="ph0", bufs=5) as p0,
                tc.tile_pool(name="ph0b", bufs=3) as p0b,
                tc.tile_pool(name="hTpool", bufs=1) as hpool,
                tc.tile_pool(name="ph0psum", bufs=3, space="PSUM") as pp_tp,
                tc.tile_pool(name="ph0psum2", bufs=3, space="PSUM") as pp_mm,
                tc.tile_pool(name="ph0psum3", bufs=2, space="PSUM") as pp_n2,
            ):
                hT = hpool.tile([128, N], F32)     # unnormalized, [hid, j]

                for c in range(NCH):
                    xT_sb = [p0.tile([128, 512], F32, tag=f"xT{fb}", name=f"xT{fb}") for fb in range(2)]
                    _transpose_x_chunk(nc, p0, pp_tp, ident, x_t, c * CH, xT_sb)
                    hp = pp_mm.tile([128, 512], F32, tag="hmm")
                    nc.tensor.matmul(hp[:], wp_sb[:, 0, :], xT_sb[0][:], start=True, stop=False)
                    nc.tensor.matmul(hp[:], wp_sb[:, 1, :], xT_sb[1][:], start=False, stop=True)
                    sl = slice(c * CH, (c + 1) * CH)
                    nc.scalar.activation(
                        hT[:, sl], hp[:], mybir.ActivationFunctionType.Relu, bias=b_sb[:]
                    )
                    # row norms^2 via ones-matmul over the hid (partition) dim
                    sq = p0b.tile([128, 512], F32, tag="sq")
                    nc.vector.tensor_mul(sq[:], hT[:, sl], hT[:, sl])
                    n2 = pp_n2.tile([1, 512], F32, tag="n2")
                    nc.tensor.matmul(n2[:], ones_col[:], sq[:], start=True, stop=True)
                    rchunk = p0b.tile([1, 512], F32, tag="rchunk")
                    nc.scalar.activation(
                        rchunk[:], n2[:], mybir.ActivationFunctionType.Sqrt
                    )
                    nc.vector.tensor_scalar_max(rchunk[:], rchunk[:], 1e-12)
                    nc.vector.reciprocal(rchunk[:], rchunk[:])
                    rb = pp_mm.tile([128, 512], F32, tag="hmm", name="rb")
                    nc.tensor.matmul(rb[:], ones_row[:], rchunk[:], start=True, stop=True)
                    nc.vector.tensor_mul(hnT[:, sl], hT[:, sl], rb[:])

                # support = h @ W_conv, row-major bf16: supp[:, jt, :]
                for jt in range(JT):
                    sp = pp_tp.tile([128, 512], F32, tag="xtp", name="sp")
                    nc.tensor.matmul(
                        sp[:, :HID], hT[:, jt * 128 : (jt + 1) * 128], wc_sb[:], start=True, stop=True
                    )
                    nc.scalar.copy(supp[:, jt, :], sp[:, :HID])

                # local (query) columns: recompute h for own rows from xloc
                for c in range(2):
                    xT_sb = [p0.tile([128, 512], F32, tag=f"xT{fb}", name=f"xT{fb}") for fb in range(2)]
                    _transpose_x_chunk(nc, p0, pp_tp, ident, xloc_t, c * CH, xT_sb)
                    hp = pp_mm.tile([128, 512], F32, tag="hmm")
                    nc.tensor.matmul(hp[:], wp_sb[:, 0, :], xT_sb[0][:], start=True, stop=False)
                    nc.tensor.matmul(hp[:], wp_sb[:, 1, :], xT_sb[1][:], start=False, stop=True)
                    nc.scalar.activation(
                        hTloc[:, c * CH : (c + 1) * CH],
                        hp[:],
                        mybir.ActivationFunctionType.Relu,
                        bias=b_sb[:],
                    )

            # ---------------- phases 1+2, grouped ----------------
            with (
                tc.tile_pool(name="zpsum", bufs=3, space="PSUM") as zp,
                tc.tile_pool(name="zsb", bufs=2) as zpool,
                tc.tile_pool(name="small", bufs=4) as sm,
                tc.tile_pool(name="masks", bufs=2 * GROUP) as mpool,
                tc.tile_pool(name="atpsum", bufs=3, space="PSUM") as atp,
                tc.tile_pool(name="atsb", bufs=6) as atsb,
                tc.tile_pool(name="opsum", bufs=1, space="PSUM") as op,
                tc.tile_pool(name="outsb", bufs=2) as osb,
            ):
                for g in range(RT // GROUP):
                    gmasks = []
                    for rt_in in range(GROUP):
                        rt = g * GROUP + rt_in
                        isl = slice(rt * 128, (rt + 1) * 128)
                        # Z row-tile: [128 i, 8192 j]
                        z_sb = zpool.tile([128, N], F32, tag="z")
                        m8 = sm.tile([128, NCH, 8], F32, tag="m8")
                        for c in range(NCH):
                            zps = zp.tile([128, 512], F32, tag="zp")
                            nc.tensor.matmul(
                                zps[:], hTloc[:, isl], hnT[:, c * CH : (c + 1) * CH],
                                start=True, stop=True,
                            )
                            sl = slice(c * CH, (c + 1) * CH)
                            nc.scalar.copy(z_sb[:, sl], zps[:])
                            nc.vector.max(m8[:, c, :], z_sb[:, sl])
                        # mini top-16 over the 128 group winners
                        t8a = sm.tile([128, 8], F32, tag="t8a")
                        m8z = sm.tile([128, NCH, 8], F32, tag="m8z")
                        t8b = sm.tile([128, 8], F32, tag="t8b")
                        tau = sm.tile([128, 1], F32, tag="tau")
                        m8f = m8[:].rearrange("p a b -> p (a b)")
                        m8zf = m8z[:].rearrange("p a b -> p (a b)")
                        nc.vector.max(t8a[:], m8f)
                        nc.vector.match_replace(m8zf, t8a[:], m8f, NEG)
                        nc.vector.max(t8b[:], m8zf)
                        # tau = (v10 + v11)/2 ; v10 = t8b[:,1], v11 = t8b[:,2]
                        nc.vector.tensor_add(tau[:], t8b[:, 1:2], t8b[:, 2:3])
                        nc.vector.tensor_scalar_mul(tau[:], tau[:], 0.5)
                        mask = mpool.tile([128, N], BF16, tag="mask")
                        nc.vector.tensor_scalar(
                            mask[:], z_sb[:], tau[:], None, op0=mybir.AluOpType.is_ge
                        )
                        gmasks.append(mask)

                    # phase 2: out rows for this group of 4 row-tiles
                    oTa = op.tile([128, GROUP * 128], F32, tag="oTa")
                    oTb = op.tile([128, GROUP * 128], F32, tag="oTb")
                    for jt in range(JT):
                        at_ps = atp.tile([128, GROUP * 128], BF16, tag="at")
                        for rt_in in range(GROUP):
                            nc.tensor.transpose(
                                at_ps[:, rt_in * 128 : (rt_in + 1) * 128],
                                gmasks[rt_in][:, jt * 128 : (jt + 1) * 128],
                                identb[:],
                            )
                        at_s = atsb.tile([128, GROUP * 128], BF16, tag="ats")
                        if jt % 2 == 0:
                            nc.vector.tensor_copy(at_s[:], at_ps[:])
                        else:
                            nc.scalar.copy(at_s[:], at_ps[:])
                        acc = oTa if jt % 2 == 0 else oTb
                        nc.tensor.matmul(
                            acc[:], supp[:, jt, :], at_s[:],
                            start=(jt < 2), stop=(jt >= JT - 2),
                        )
                    # relu(0.1 * (oTa + oTb)), transpose back to [i, hid], DMA out
                    ob_sb = osb.tile([128, GROUP * 128], F32, tag="obsb")
                    nc.scalar.copy(ob_sb[:], oTb[:])
                    osum = osb.tile([128, GROUP * 128], F32, tag="osum")
                    nc.vector.tensor_add(osum[:], oTa[:], ob_sb[:])
                    oT_sb = osb.tile([128, GROUP * 128], F32, tag="oTsb")
                    nc.scalar.activation(
                        oT_sb[:], osum[:], mybir.ActivationFunctionType.Relu, scale=0.1
                    )
                    for rt_in in range(GROUP):
                        ops_ = atp.tile([128, GROUP * 128], F32, tag="at", name="ops_")
                        nc.tensor.transpose(
                            ops_[:, :128], oT_sb[:, rt_in * 128 : (rt_in + 1) * 128], ident[:]
                        )
                        o_sb = osb.tile([128, 128], F32, tag="osb")
                        nc.scalar.copy(o_sb[:], ops_[:, :128])
                        r0 = (g * GROUP + rt_in) * 128
                        nc.sync.dma_start(out_t[r0 : r0 + 128, :], o_sb[:])

    nc.compile()
    return nc


_NC_CACHE = {}


def kernel(x, W_proj, b_proj, W_conv):
    if "nc" not in _NC_CACHE:
        _NC_CACHE["nc"] = build_nc()
    nc = _NC_CACHE["nc"]
    x = np.ascontiguousarray(x, dtype=np.float32)
    in_maps = []
    for c in range(NCORES):
        in_maps.append(
            {
                "x": x,
                "xloc": np.ascontiguousarray(x[c * NLOC : (c + 1) * NLOC]),
                "W_proj": np.ascontiguousarray(W_proj, dtype=np.float32),
                "b_proj": np.ascontiguousarray(b_proj, dtype=np.float32),
                "W_conv": np.ascontiguousarray(W_conv, dtype=np.float32),
            }
        )
    res = run_bass_kernel_spmd(nc, in_maps, core_ids=list(range(NCORES)))
    return np.concatenate([res.results[c]["out"] for c in range(NCORES)], axis=0)



# revision 2
# speedup vs baseline: 1.0006x; 1.0006x over previous
"""Trainium2 Bass kernel for LANLayer-style GNN message passing (v2).

Reference (N=8192, DIM=256, HID=128, K=10):
    h = relu(x @ W_proj + b); hn = h / ||h||
    sim = (hn @ hn.T + 1)/2; probs = softmax(sim/T); topi = top_k(probs, 10)
    A_hat = one_hot(topi) - diag + eye  (exactly 10 ones/row)
    out = relu((A_hat/10) @ (h @ W_conv))

Facts used:
  - top-k of softmax(sim) rows == top-k of z[i,j] = hn_i . hn_j; the diagonal
    is always rank-1; so A_hat row = {z >= tau}, tau = (rank10+rank11)/2.
  - every row degree is exactly 10 -> adj_norm = A_hat/10.

Per core (input rows rotated so the core's 1024 rows are columns 0-1023):
  P0: h = relu(x@Wp+b) in fp32 (selection accuracy), per-chunk norms from
      fp16 squares via ones-matmul, r = 1/sqrt on partition 0, Pool
      partition_broadcast + DVE mult -> fp16 keys hnT16. supp^T = Wc^T @ h16
      (fp16), chunks in the ACT-Sign range pre-scaled by 0.5 with their
      column-sums accumulated (mask sign-correction bias); DMA-transposed to
      row-major supp.
  P1 software-pipelined per 128-row tile rt:
      iter rt: z matmuls (fp16, fp32 PSUM) + PSUM->SBUF copies (ACT/Pool);
               per-2048 top8 + mini top-16 -> tau(rt)  [DVE]
               cmp(rt-1): ACT Sign(z-tau) on [0,4608), Pool is_ge on rest
               mask(rt-1) DMA-transpose (16x128 XBAR) -> maskT
               p2(rt-2): 64 accumulating fp16 matmuls outT = supp^T @ maskT
               finish(rt-2): relu(0.1 acc + 0.1 cs_half) -> PE transpose ->
               fp16 DMA out.
"""

import numpy as np

import concourse.bass as bass
import concourse.mybir as mybir
import concourse.tile as tile
from concourse import bacc
from concourse.bass_utils import run_bass_kernel_spmd
from concourse.masks import make_identity

N, DIM, HID = 8192, 256, 128
NCORES = 8
NLOC = N // NCORES          # 1024 rows per core
RT = NLOC // 128            # 8 row-tiles per core
CH = 512                    # phase-0 column chunk
NCH = N // CH               # 16
ZC = 1024                   # z psum/copy chunk
NZC = N // ZC               # 8
JT = N // 128               # 64
F32 = mybir.dt.float32
F16 = mybir.dt.float16
NEG = -1.0e30

SIGN_HI = 2048                      # ACT Sign range [0, SIGN_HI), Pool rest
COPY_ENG = ["A"] * 8                # per ZC chunk (Pool cannot read PSUM)


def build_nc():
    nc = bacc.Bacc(None, target_bir_lowering=False)

    xh_t = nc.dram_tensor("xTh", [128, 2, N], F16, kind="ExternalInput")
    xl_t = nc.dram_tensor("xTl", [128, 2, N], F16, kind="ExternalInput")
    wph_t = nc.dram_tensor("W_proj_h", [128, 2, HID], F16, kind="ExternalInput")
    wpl_t = nc.dram_tensor("W_proj_l", [128, 2, HID], F16, kind="ExternalInput")
    bp_t = nc.dram_tensor("b_proj", [128, 1], F32, kind="ExternalInput")
    wc_t = nc.dram_tensor("W_conv", [128, HID], F32, kind="ExternalInput")
    out_t = nc.dram_tensor("out", [NLOC, HID], F16, kind="ExternalOutput")

    with tile.TileContext(nc) as tc:
        with (
            tc.tile_pool(name="const", bufs=1) as cpool,
            tc.tile_pool(name="persist", bufs=1) as pers,
        ):
            ident16 = cpool.tile([128, 128], F16)
            make_identity(nc, ident16[:])
            ones16 = cpool.tile([128, 1], F16)
            nc.vector.memset(ones16[:], 1.0)
            two_ = cpool.tile([128, 1], F32)
            nc.vector.memset(two_[:], 2.0)
            wph_sb = cpool.tile([128, 2, HID], F16)
            nc.sync.dma_start(wph_sb[:], wph_t[:])
            wpl_sb = cpool.tile([128, 2, HID], F16)
            nc.sync.dma_start(wpl_sb[:], wpl_t[:])
            b_sb = cpool.tile([128, 1], F32)
            nc.sync.dma_start(b_sb[:], bp_t[:])
            wc_sb = cpool.tile([128, HID], F32)
            nc.sync.dma_start(wc_sb[:], wc_t[:])
            wc16 = cpool.tile([128, HID], F16)
            nc.vector.tensor_copy(wc16[:], wc_sb[:])

            hnT16 = pers.tile([128, N], F16)             # keys; queries = cols 0-1023
            supp = pers.tile([128, JT, HID], F16)        # supp rows [j%128, jt, hid]
            csb = pers.tile([128, 1], F32)               # 0.1 * cs_half bias

            # ---------------- phase 0 ----------------
            with (
                tc.tile_pool(name="xpool", bufs=2) as xpool,
                tc.tile_pool(name="hpool", bufs=1) as hpool,
                tc.tile_pool(name="p0sb", bufs=6) as p0sb,
                tc.tile_pool(name="p0norm", bufs=1) as p0n,
                tc.tile_pool(name="hppsum", bufs=3, space="PSUM") as hpp,
                tc.tile_pool(name="n2psum", bufs=3, space="PSUM") as n2p,
                tc.tile_pool(name="sppsum", bufs=2, space="PSUM") as spp,
            ):
                hT = hpool.tile([128, N], F32)
                h16 = hpool.tile([128, N], F16)
                suppT16 = hpool.tile([128, N], F16)
                su = p0n.tile([128, NCH], F32)

                for c in range(NCH):
                    csl = slice(c * CH, (c + 1) * CH)
                    if c % 2 == 0:
                        xc = xpool.tile([128, 2, ZC], F16, tag="xc")
                        nc.sync.dma_start(xc[:], xh_t[:, :, c * CH : (c + 2) * CH])
                        xcl = xpool.tile([128, 2, ZC], F16, tag="xcl")
                        nc.sync.dma_start(xcl[:], xl_t[:, :, c * CH : (c + 2) * CH])
                    xs = xc[:, :, (c % 2) * CH : ((c % 2) + 1) * CH]
                    xsl = xcl[:, :, (c % 2) * CH : ((c % 2) + 1) * CH]
                    hp = hpp.tile([128, CH], F32, tag="hp")
                    nc.tensor.matmul(hp[:], wph_sb[:, 0, :], xs[:, 0, :], start=True, stop=False)
                    nc.tensor.matmul(hp[:], wph_sb[:, 1, :], xs[:, 1, :], start=False, stop=False)
                    nc.tensor.matmul(hp[:], wpl_sb[:, 0, :], xs[:, 0, :], start=False, stop=False)
                    nc.tensor.matmul(hp[:], wpl_sb[:, 1, :], xs[:, 1, :], start=False, stop=False)
                    nc.tensor.matmul(hp[:], wph_sb[:, 0, :], xsl[:, 0, :], start=False, stop=False)
                    nc.tensor.matmul(hp[:], wph_sb[:, 1, :], xsl[:, 1, :], start=False, stop=True)
                    nc.scalar.activation(
                        hT[:, csl], hp[:], mybir.ActivationFunctionType.Relu, bias=b_sb[:]
                    )
                    nc.vector.tensor_copy(h16[:, csl], hT[:, csl])
                    sq = p0sb.tile([128, CH], F16, tag="sq")
                    nc.vector.tensor_mul(sq[:], h16[:, csl], h16[:, csl])
                    n2 = n2p.tile([1, CH], F32, tag="n2")
                    nc.tensor.matmul(n2[:], ones16[:], sq[:], start=True, stop=True)
                    s_ = p0sb.tile([1, CH], F32, tag="s_")
                    nc.scalar.activation(s_[:], n2[:], mybir.ActivationFunctionType.Sqrt)
                    rc = p0sb.tile([1, CH], F32, tag="rc")
                    nc.vector.reciprocal(rc[:], s_[:])
                    rb = p0sb.tile([128, CH], F32, tag="rb")
                    nc.gpsimd.partition_broadcast(rb[:], rc[:])
                    nc.vector.tensor_mul(hnT16[:, csl], hT[:, csl], rb[:])
                    sp = spp.tile([128, CH], F32, tag="sp")
                    nc.tensor.matmul(sp[:], wc16[:], h16[:, csl], start=True, stop=True)
                    halved = c * CH < SIGN_HI
                    nc.scalar.activation(
                        suppT16[:, csl], sp[:], mybir.ActivationFunctionType.Copy,
                        scale=0.5 if halved else 1.0,
                        accum_out=su[:, c : c + 1],
                    )

                nc.sync.dma_start(supp[:], suppT16[:], transpose=True)
                su_sum = p0n.tile([128, 1], F32)
                nc.vector.tensor_reduce(
                    su_sum[:], su[:, : SIGN_HI // CH],
                    mybir.AxisListType.X, mybir.AluOpType.add,
                )
                nc.vector.tensor_scalar_mul(csb[:], su_sum[:], 0.1)

            # ---------------- phases 1+2, software pipelined ----------------
            with (
                tc.tile_pool(name="zsb", bufs=3) as zsbp,
                tc.tile_pool(name="maskp", bufs=2) as maskp,
                tc.tile_pool(name="masktp", bufs=2) as masktp,
                tc.tile_pool(name="small", bufs=4) as sm,
                tc.tile_pool(name="outsb", bufs=2) as osb,
                tc.tile_pool(name="zpsum", bufs=3, space="PSUM") as zp,
                tc.tile_pool(name="apsum", bufs=2, space="PSUM") as ap,
            ):
                state = {}   # rt -> dict(z_sb, tau, ntau, mask, maskT)

                def stage_z(rt, pe_filler=None):
                    qsl = slice(rt * 128, (rt + 1) * 128)
                    z_sb = zsbp.tile([128, N], F32, tag="z")
                    for c in range(NZC):
                        zps = zp.tile([128, ZC], F32, tag="zp")
                        for half in range(2):
                            hsl = slice(half * CH, (half + 1) * CH)
                            jsl = slice(c * ZC + half * CH, c * ZC + (half + 1) * CH)
                            nc.tensor.matmul(
                                zps[:, hsl], hnT16[:, qsl], hnT16[:, jsl],
                                start=True, stop=True,
                            )
                        zsl = slice(c * ZC, (c + 1) * ZC)
                        if COPY_ENG[c] == "A":
                            nc.scalar.copy(z_sb[:, zsl], zps[:])
                        elif COPY_ENG[c] == "P":
                            nc.gpsimd.tensor_copy(z_sb[:, zsl], zps[:])
                        else:
                            nc.vector.tensor_copy(z_sb[:, zsl], zps[:])
                        if pe_filler is not None and c >= NZC // 2:
                            for _ in range(JT // (NZC // 2)):
                                next(pe_filler, None)
                    state[rt] = {"z": z_sb}

                def stage_tau(rt):
                    st = state[rt]
                    z_sb = st["z"]
                    m8 = sm.tile([128, 4, 8], F32, tag="m8")
                    for k in range(4):
                        nc.vector.max(m8[:, k, :], z_sb[:, k * 2048 : (k + 1) * 2048])
                    m8f = m8[:].rearrange("p a b -> p (a b)")
                    t8a = sm.tile([128, 8], F32, tag="t8a")
                    nc.vector.max(t8a[:], m8f)
                    m8z = sm.tile([128, 32], F32, tag="m8z")
                    nc.vector.match_replace(m8z[:], t8a[:], m8f, NEG)
                    t8b = sm.tile([128, 8], F32, tag="t8b")
                    nc.vector.max(t8b[:], m8z[:])
                    tsum = sm.tile([128, 1], F32, tag="tsum")
                    nc.vector.tensor_add(tsum[:], t8b[:, 1:2], t8b[:, 2:3])
                    tau = sm.tile([128, 1], F32, tag="tau")
                    nc.vector.tensor_scalar_mul(tau[:], tsum[:], 0.5)
                    ntau = sm.tile([128, 1], F32, tag="ntau")
                    nc.vector.tensor_scalar_mul(ntau[:], tsum[:], -0.5)
                    st["tau"], st["ntau"] = tau, ntau

                def stage_cmp(rt):
                    st = state[rt]
                    z_sb, tau, ntau = st["z"], st["tau"], st["ntau"]
                    tail = rt >= RT - 2
                    st["tail"] = tail
                    mask = maskp.tile([128, N], F16, tag="mask")
                    maskT = masktp.tile([128, JT, 128], F16, tag="maskT")
                    for k in range(4):
                        ksl = slice(k * 2048, (k + 1) * 2048)
                        if (k + 1) * 2048 <= SIGN_HI:
                            if tail:
                                # {0,2} mask on the pre-halved range: no bias needed
                                nc.vector.tensor_scalar(
                                    mask[:, ksl], z_sb[:, ksl], tau[:], two_[:],
                                    op0=mybir.AluOpType.is_ge,
                                    op1=mybir.AluOpType.mult,
                                )
                            else:
                                nc.scalar.activation(
                                    mask[:, ksl], z_sb[:, ksl],
                                    mybir.ActivationFunctionType.Sign, bias=ntau[:],
                                )
                        elif tail and k == 1:
                            nc.vector.tensor_scalar(
                                mask[:, ksl], z_sb[:, ksl], tau[:], None,
                                op0=mybir.AluOpType.is_ge,
                            )
                        else:
                            nc.gpsimd.tensor_scalar(
                                mask[:, ksl], z_sb[:, ksl], tau[:], None,
                                op0=mybir.AluOpType.is_ge,
                            )
                        nc.sync.dma_start(
                            maskT[:, k * 16 : (k + 1) * 16, :], mask[:, ksl],
                            transpose=True,
                        )
                    st["maskT"] = maskT

                def p2_mms(rt):
                    st = state[rt]
                    maskT = st["maskT"]
                    acc = ap.tile([128, 128], F32, tag="acc")
                    st["acc"] = acc
                    for jt in range(JT):
                        nc.tensor.matmul(
                            acc[:], supp[:, jt, :], maskT[:, jt, :],
                            start=(jt == 0), stop=(jt == JT - 1),
                        )
                        yield

                def stage_p2_finish(rt):
                    st = state[rt]
                    acc = st["acc"]
                    oT = osb.tile([128, 128], F16, tag="oT")
                    nc.scalar.activation(
                        oT[:], acc[:], mybir.ActivationFunctionType.Relu,
                        bias=0.0 if st["tail"] else csb[:], scale=0.1,
                    )
                    o_sb = osb.tile([128, 128], F16, tag="osb")
                    nc.scalar.dma_start(o_sb[:], oT[:], transpose=True)
                    nc.scalar.dma_start(out_t[rt * 128 : (rt + 1) * 128, :], o_sb[:])
                    del state[rt]

                for rt in range(RT):
                    filler = p2_mms(rt - 2) if rt >= 2 else None
                    stage_z(rt, filler)
                    if filler is not None:
                        for _ in filler:
                            pass
                        stage_p2_finish(rt - 2)
                    stage_tau(rt)
                    if rt >= 1:
                        stage_cmp(rt - 1)
                stage_cmp(RT - 1)
                for rt in (RT - 2, RT - 1):
                    for _ in p2_mms(rt):
                        pass
                    stage_p2_finish(rt)

    nc.compile()
    return nc


_NC_CACHE = {}


def kernel(x, W_proj, b_proj, W_conv):
    if "nc" not in _NC_CACHE:
        _NC_CACHE["nc"] = build_nc()
    nc = _NC_CACHE["nc"]
    x = np.ascontiguousarray(x, dtype=np.float32)
    W_proj = np.ascontiguousarray(W_proj, dtype=np.float32)
    b_proj = np.ascontiguousarray(b_proj, dtype=np.float32)
    W_conv = np.ascontiguousarray(W_conv, dtype=np.float32)

    wp3 = W_proj.reshape(2, 128, HID).transpose(1, 0, 2)
    wph = wp3.astype(np.float16)
    wpl = (wp3 - wph.astype(np.float32)).astype(np.float16)
    b_dev = np.ascontiguousarray(b_proj.reshape(HID, 1))
    in_maps = []
    for c in range(NCORES):
        x_rot = np.concatenate([x[c * NLOC :], x[: c * NLOC]], axis=0)
        xT = np.ascontiguousarray(x_rot.T)            # [256, 8192]
        xT3 = xT.reshape(2, 128, N).transpose(1, 0, 2)
        xh = np.ascontiguousarray(xT3.astype(np.float16))
        xl = np.ascontiguousarray((xT3 - xh.astype(np.float32)).astype(np.float16))
        in_maps.append(
            {"xTh": xh, "xTl": xl, "W_proj_h": np.ascontiguousarray(wph),
             "W_proj_l": np.ascontiguousarray(wpl), "b_proj": b_dev, "W_conv": W_conv}
        )
    res = run_bass_kernel_spmd(nc, in_maps, core_ids=list(range(NCORES)))
    return np.concatenate(
        [res.results[c]["out"].astype(np.float32) for c in range(NCORES)], axis=0
    )


# revision 3
# speedup vs baseline: 1.0526x; 1.0519x over previous
"""Trainium2 Bass kernel for LANLayer-style GNN message passing (v2).

Reference (N=8192, DIM=256, HID=128, K=10):
    h = relu(x @ W_proj + b); hn = h / ||h||
    sim = (hn @ hn.T + 1)/2; probs = softmax(sim/T); topi = top_k(probs, 10)
    A_hat = one_hot(topi) - diag + eye  (exactly 10 ones/row)
    out = relu((A_hat/10) @ (h @ W_conv))

Facts used:
  - top-k of softmax(sim) rows == top-k of z[i,j] = hn_i . hn_j; the diagonal
    is always rank-1; so A_hat row = {z >= tau}, tau = (rank10+rank11)/2.
  - every row degree is exactly 10 -> adj_norm = A_hat/10.

Per core (input rows rotated so the core's 1024 rows are columns 0-1023):
  P0: h = relu(x@Wp+b) in fp32 (selection accuracy), per-chunk norms from
      fp16 squares via ones-matmul, r = 1/sqrt on partition 0, Pool
      partition_broadcast + DVE mult -> fp16 keys hnT16. supp^T = Wc^T @ h16
      (fp16), chunks in the ACT-Sign range pre-scaled by 0.5 with their
      column-sums accumulated (mask sign-correction bias); DMA-transposed to
      row-major supp.
  P1 software-pipelined per 128-row tile rt:
      iter rt: z matmuls (fp16, fp32 PSUM) + PSUM->SBUF copies (ACT/Pool);
               per-2048 top8 + mini top-16 -> tau(rt)  [DVE]
               cmp(rt-1): ACT Sign(z-tau) on [0,4608), Pool is_ge on rest
               mask(rt-1) DMA-transpose (16x128 XBAR) -> maskT
               p2(rt-2): 64 accumulating fp16 matmuls outT = supp^T @ maskT
               finish(rt-2): relu(0.1 acc + 0.1 cs_half) -> PE transpose ->
               fp16 DMA out.
"""

import numpy as np

import concourse.bass as bass
import concourse.mybir as mybir
import concourse.tile as tile
from concourse import bacc
from concourse.bass_utils import run_bass_kernel_spmd
from concourse.masks import make_identity

N, DIM, HID = 8192, 256, 128
NCORES = 8
NLOC = N // NCORES          # 1024 rows per core
RT = NLOC // 128            # 8 row-tiles per core
CH = 512                    # phase-0 column chunk
NCH = N // CH               # 16
ZC = 1024                   # z psum/copy chunk
NZC = N // ZC               # 8
JT = N // 128               # 64
F32 = mybir.dt.float32
F16 = mybir.dt.float16
NEG = -1.0e30

SIGN_HI = 2048                      # ACT Sign range [0, SIGN_HI), Pool rest
COPY_ENG = ["A"] * 8                # per ZC chunk (Pool cannot read PSUM)


def build_nc():
    nc = bacc.Bacc(None, target_bir_lowering=False)

    xh_t = nc.dram_tensor("xTh", [128, 2, N], F16, kind="ExternalInput")
    xl_t = nc.dram_tensor("xTl", [128, 2, N], F16, kind="ExternalInput")
    wph_t = nc.dram_tensor("W_proj_h", [128, 2, HID], F16, kind="ExternalInput")
    wpl_t = nc.dram_tensor("W_proj_l", [128, 2, HID], F16, kind="ExternalInput")
    bp_t = nc.dram_tensor("b_proj", [128, 1], F32, kind="ExternalInput")
    wc_t = nc.dram_tensor("W_conv", [128, HID], F32, kind="ExternalInput")
    out_t = nc.dram_tensor("out", [NLOC, HID], F16, kind="ExternalOutput")

    with tile.TileContext(nc) as tc:
        with (
            tc.tile_pool(name="const", bufs=1) as cpool,
            tc.tile_pool(name="persist", bufs=1) as pers,
        ):
            ones16 = cpool.tile([128, 1], F16)
            nc.vector.memset(ones16[:], 1.0)
            two_ = cpool.tile([128, 1], F32)
            nc.vector.memset(two_[:], 2.0)
            wph_sb = cpool.tile([128, 2, HID], F16)
            nc.sync.dma_start(wph_sb[:], wph_t[:])
            wpl_sb = cpool.tile([128, 2, HID], F16)
            nc.sync.dma_start(wpl_sb[:], wpl_t[:])
            b_sb = cpool.tile([128, 1], F32)
            nc.sync.dma_start(b_sb[:], bp_t[:])
            wc_sb = cpool.tile([128, HID], F32)
            nc.sync.dma_start(wc_sb[:], wc_t[:])
            wc16 = cpool.tile([128, HID], F16)
            nc.vector.tensor_copy(wc16[:], wc_sb[:])

            hnT16 = pers.tile([128, N], F16)             # keys; queries = cols 0-1023
            supp = pers.tile([128, JT, HID], F16)        # supp rows [j%128, jt, hid]
            csb = pers.tile([128, 1], F32)               # 0.1 * cs_half bias

            # ---------------- phase 0 ----------------
            with (
                tc.tile_pool(name="xpool", bufs=2) as xpool,
                tc.tile_pool(name="hpool", bufs=1) as hpool,
                tc.tile_pool(name="p0sb", bufs=6) as p0sb,
                tc.tile_pool(name="p0norm", bufs=1) as p0n,
                tc.tile_pool(name="hppsum", bufs=3, space="PSUM") as hpp,
                tc.tile_pool(name="n2psum", bufs=3, space="PSUM") as n2p,
                tc.tile_pool(name="sppsum", bufs=2, space="PSUM") as spp,
            ):
                hT = hpool.tile([128, N], F32)
                h16 = hpool.tile([128, N], F16)
                suppT16 = hpool.tile([128, N], F16)
                su = p0n.tile([128, NCH], F32)

                for c in range(NCH):
                    csl = slice(c * CH, (c + 1) * CH)
                    if c % 2 == 0:
                        xc = xpool.tile([128, 2, ZC], F16, tag="xc")
                        nc.sync.dma_start(xc[:], xh_t[:, :, c * CH : (c + 2) * CH])
                        xcl = xpool.tile([128, 2, ZC], F16, tag="xcl")
                        nc.sync.dma_start(xcl[:], xl_t[:, :, c * CH : (c + 2) * CH])
                    xs = xc[:, :, (c % 2) * CH : ((c % 2) + 1) * CH]
                    xsl = xcl[:, :, (c % 2) * CH : ((c % 2) + 1) * CH]
                    hp = hpp.tile([128, CH], F32, tag="hp")
                    nc.tensor.matmul(hp[:], wph_sb[:, 0, :], xs[:, 0, :], start=True, stop=False)
                    nc.tensor.matmul(hp[:], wph_sb[:, 1, :], xs[:, 1, :], start=False, stop=False)
                    nc.tensor.matmul(hp[:], wpl_sb[:, 0, :], xs[:, 0, :], start=False, stop=False)
                    nc.tensor.matmul(hp[:], wpl_sb[:, 1, :], xs[:, 1, :], start=False, stop=False)
                    nc.tensor.matmul(hp[:], wph_sb[:, 0, :], xsl[:, 0, :], start=False, stop=False)
                    nc.tensor.matmul(hp[:], wph_sb[:, 1, :], xsl[:, 1, :], start=False, stop=True)
                    nc.scalar.activation(
                        hT[:, csl], hp[:], mybir.ActivationFunctionType.Relu, bias=b_sb[:]
                    )
                    nc.vector.tensor_copy(h16[:, csl], hT[:, csl])
                    sq = p0sb.tile([128, CH], F16, tag="sq")
                    nc.vector.tensor_mul(sq[:], h16[:, csl], h16[:, csl])
                    n2 = n2p.tile([1, CH], F32, tag="n2")
                    nc.tensor.matmul(n2[:], ones16[:], sq[:], start=True, stop=True)
                    s_ = p0sb.tile([1, CH], F32, tag="s_")
                    nc.scalar.activation(s_[:], n2[:], mybir.ActivationFunctionType.Sqrt)
                    rc = p0sb.tile([1, CH], F32, tag="rc")
                    nc.vector.reciprocal(rc[:], s_[:])
                    rb = p0sb.tile([128, CH], F32, tag="rb")
                    nc.gpsimd.partition_broadcast(rb[:], rc[:])
                    nc.vector.tensor_mul(hnT16[:, csl], hT[:, csl], rb[:])
                    sp = spp.tile([128, CH], F32, tag="sp")
                    nc.tensor.matmul(sp[:], wc16[:], h16[:, csl], start=True, stop=True)
                    halved = c * CH < SIGN_HI
                    nc.scalar.activation(
                        suppT16[:, csl], sp[:], mybir.ActivationFunctionType.Copy,
                        scale=0.5 if halved else 1.0,
                        accum_out=su[:, c : c + 1],
                    )
                    if c % 4 == 3:
                        k = c // 4
                        nc.sync.dma_start(
                            supp[:, k * 16 : (k + 1) * 16, :],
                            suppT16[:, k * 2048 : (k + 1) * 2048],
                            transpose=True,
                        )
                su_sum = p0n.tile([128, 1], F32)
                nc.vector.tensor_reduce(
                    su_sum[:], su[:, : SIGN_HI // CH],
                    mybir.AxisListType.X, mybir.AluOpType.add,
                )
                nc.vector.tensor_scalar_mul(csb[:], su_sum[:], 0.1)

            # ---------------- phases 1+2, software pipelined ----------------
            with (
                tc.tile_pool(name="zsb", bufs=3) as zsbp,
                tc.tile_pool(name="maskp", bufs=2) as maskp,
                tc.tile_pool(name="masktp", bufs=2) as masktp,
                tc.tile_pool(name="small", bufs=4) as sm,
                tc.tile_pool(name="outsb", bufs=3) as osb,
                tc.tile_pool(name="zpsum", bufs=3, space="PSUM") as zp,
                tc.tile_pool(name="apsum", bufs=2, space="PSUM") as ap,
            ):
                state = {}   # rt -> dict(z_sb, tau, ntau, mask, maskT)

                def stage_z(rt, pe_filler=None):
                    qsl = slice(rt * 128, (rt + 1) * 128)
                    z_sb = zsbp.tile([128, N], F32, tag="z")
                    m8p = None
                    if rt == RT - 1 or rt == 0:
                        m8p = sm.tile([128, NZC, 8], F32, tag="m8p")
                    for c in range(NZC):
                        zps = zp.tile([128, ZC], F32, tag="zp")
                        for half in range(2):
                            hsl = slice(half * CH, (half + 1) * CH)
                            jsl = slice(c * ZC + half * CH, c * ZC + (half + 1) * CH)
                            nc.tensor.matmul(
                                zps[:, hsl], hnT16[:, qsl], hnT16[:, jsl],
                                start=True, stop=True,
                            )
                        if m8p is not None:
                            nc.vector.max(m8p[:, c, :], zps[:])
                        zsl = slice(c * ZC, (c + 1) * ZC)
                        if COPY_ENG[c] == "A":
                            nc.scalar.copy(z_sb[:, zsl], zps[:])
                        elif COPY_ENG[c] == "P":
                            nc.gpsimd.tensor_copy(z_sb[:, zsl], zps[:])
                        else:
                            nc.vector.tensor_copy(z_sb[:, zsl], zps[:])
                        if pe_filler is not None and c >= NZC // 2:
                            for _ in range(JT // (NZC // 2)):
                                next(pe_filler, None)
                    state[rt] = {"z": z_sb, "m8p": m8p}

                def stage_tau(rt):
                    st = state[rt]
                    z_sb = st["z"]
                    if st["m8p"] is not None:
                        m8f = st["m8p"][:].rearrange("p a b -> p (a b)")
                        nm8 = NZC * 8
                    else:
                        m8 = sm.tile([128, 4, 8], F32, tag="m8")
                        for k in range(4):
                            nc.vector.max(m8[:, k, :], z_sb[:, k * 2048 : (k + 1) * 2048])
                        m8f = m8[:].rearrange("p a b -> p (a b)")
                        nm8 = 32
                    t8a = sm.tile([128, 8], F32, tag="t8a")
                    nc.vector.max(t8a[:], m8f)
                    m8z = sm.tile([128, 64], F32, tag="m8z")
                    nc.vector.match_replace(m8z[:, :nm8], t8a[:], m8f, NEG)
                    t8b = sm.tile([128, 8], F32, tag="t8b")
                    nc.vector.max(t8b[:], m8z[:, :nm8])
                    tsum = sm.tile([128, 1], F32, tag="tsum")
                    nc.vector.tensor_add(tsum[:], t8b[:, 1:2], t8b[:, 2:3])
                    tau = sm.tile([128, 1], F32, tag="tau")
                    nc.vector.tensor_scalar_mul(tau[:], tsum[:], 0.5)
                    ntau = sm.tile([128, 1], F32, tag="ntau")
                    nc.vector.tensor_scalar_mul(ntau[:], tsum[:], -0.5)
                    st["tau"], st["ntau"] = tau, ntau

                def stage_cmp(rt):
                    st = state[rt]
                    z_sb, tau, ntau = st["z"], st["tau"], st["ntau"]
                    tail = rt >= RT - 2
                    st["tail"] = tail
                    mask = maskp.tile([128, N], F16, tag="mask")
                    maskT = masktp.tile([128, JT, 128], F16, tag="maskT")
                    for k in range(4):
                        ksl = slice(k * 2048, (k + 1) * 2048)
                        if (k + 1) * 2048 <= SIGN_HI:
                            if tail:
                                # {0,2} mask on the pre-halved range: no bias needed
                                nc.vector.tensor_scalar(
                                    mask[:, ksl], z_sb[:, ksl], tau[:], two_[:],
                                    op0=mybir.AluOpType.is_ge,
                                    op1=mybir.AluOpType.mult,
                                )
                            else:
                                nc.scalar.activation(
                                    mask[:, ksl], z_sb[:, ksl],
                                    mybir.ActivationFunctionType.Sign, bias=ntau[:],
                                )
                        elif tail and k == 1:
                            nc.vector.tensor_scalar(
                                mask[:, ksl], z_sb[:, ksl], tau[:], None,
                                op0=mybir.AluOpType.is_ge,
                            )
                        else:
                            nc.gpsimd.tensor_scalar(
                                mask[:, ksl], z_sb[:, ksl], tau[:], None,
                                op0=mybir.AluOpType.is_ge,
                            )
                        nc.sync.dma_start(
                            maskT[:, k * 16 : (k + 1) * 16, :], mask[:, ksl],
                            transpose=True,
                        )
                    st["maskT"] = maskT

                def p2_mms(rt):
                    st = state[rt]
                    maskT = st["maskT"]
                    acc = ap.tile([128, 128], F32, tag="acc")
                    st["acc"] = acc
                    for jt in range(JT):
                        nc.tensor.matmul(
                            acc[:], supp[:, jt, :], maskT[:, jt, :],
                            start=(jt == 0), stop=(jt == JT - 1),
                        )
                        yield

                def stage_p2_finish(rt):
                    st = state[rt]
                    acc = st["acc"]
                    oT = osb.tile([128, 128], F16, tag="oT")
                    nc.scalar.activation(
                        oT[:], acc[:], mybir.ActivationFunctionType.Relu,
                        bias=0.0 if st["tail"] else csb[:], scale=0.1,
                    )
                    o_sb = osb.tile([128, 128], F16, tag="osb")
                    nc.scalar.dma_start(o_sb[:], oT[:], transpose=True)
                    nc.scalar.dma_start(out_t[rt * 128 : (rt + 1) * 128, :], o_sb[:])
                    del state[rt]

                for rt in range(RT):
                    filler = p2_mms(rt - 2) if rt >= 2 else None
                    stage_z(rt, filler)
                    if filler is not None:
                        for _ in filler:
                            pass
                        stage_p2_finish(rt - 2)
                    stage_tau(rt)
                    if rt >= 1:
                        stage_cmp(rt - 1)
                stage_cmp(RT - 1)
                for rt in (RT - 2, RT - 1):
                    for _ in p2_mms(rt):
                        pass
                    stage_p2_finish(rt)

    nc.compile()
    return nc


_NC_CACHE = {}


def kernel(x, W_proj, b_proj, W_conv):
    if "nc" not in _NC_CACHE:
        _NC_CACHE["nc"] = build_nc()
    nc = _NC_CACHE["nc"]
    x = np.ascontiguousarray(x, dtype=np.float32)
    W_proj = np.ascontiguousarray(W_proj, dtype=np.float32)
    b_proj = np.ascontiguousarray(b_proj, dtype=np.float32)
    W_conv = np.ascontiguousarray(W_conv, dtype=np.float32)

    wp3 = W_proj.reshape(2, 128, HID).transpose(1, 0, 2)
    wph = wp3.astype(np.float16)
    wpl = (wp3 - wph.astype(np.float32)).astype(np.float16)
    b_dev = np.ascontiguousarray(b_proj.reshape(HID, 1))
    in_maps = []
    for c in range(NCORES):
        x_rot = np.concatenate([x[c * NLOC :], x[: c * NLOC]], axis=0)
        xT = np.ascontiguousarray(x_rot.T)            # [256, 8192]
        xT3 = xT.reshape(2, 128, N).transpose(1, 0, 2)
        xh = np.ascontiguousarray(xT3.astype(np.float16))
        xl = np.ascontiguousarray((xT3 - xh.astype(np.float32)).astype(np.float16))
        in_maps.append(
            {"xTh": xh, "xTl": xl, "W_proj_h": np.ascontiguousarray(wph),
             "W_proj_l": np.ascontiguousarray(wpl), "b_proj": b_dev, "W_conv": W_conv}
        )
    res = run_bass_kernel_spmd(nc, in_maps, core_ids=list(range(NCORES)))
    return np.concatenate(
        [res.results[c]["out"].astype(np.float32) for c in range(NCORES)], axis=0
    )


# revision 4
# speedup vs baseline: 1.0711x; 1.0176x over previous
"""Trainium2 Bass kernel for LANLayer-style GNN message passing (v2).

Reference (N=8192, DIM=256, HID=128, K=10):
    h = relu(x @ W_proj + b); hn = h / ||h||
    sim = (hn @ hn.T + 1)/2; probs = softmax(sim/T); topi = top_k(probs, 10)
    A_hat = one_hot(topi) - diag + eye  (exactly 10 ones/row)
    out = relu((A_hat/10) @ (h @ W_conv))

Facts used:
  - top-k of softmax(sim) rows == top-k of z[i,j] = hn_i . hn_j; the diagonal
    is always rank-1; so A_hat row = {z >= tau}, tau = (rank10+rank11)/2.
  - every row degree is exactly 10 -> adj_norm = A_hat/10.

Per core (input rows rotated so the core's 1024 rows are columns 0-1023):
  P0: h = relu(x@Wp+b) in fp32 (selection accuracy), per-chunk norms from
      fp16 squares via ones-matmul, r = 1/sqrt on partition 0, Pool
      partition_broadcast + DVE mult -> fp16 keys hnT16. supp^T = Wc^T @ h16
      (fp16), chunks in the ACT-Sign range pre-scaled by 0.5 with their
      column-sums accumulated (mask sign-correction bias); DMA-transposed to
      row-major supp.
  P1 software-pipelined per 128-row tile rt:
      iter rt: z matmuls (fp16, fp32 PSUM) + PSUM->SBUF copies (ACT/Pool);
               per-2048 top8 + mini top-16 -> tau(rt)  [DVE]
               cmp(rt-1): ACT Sign(z-tau) on [0,4608), Pool is_ge on rest
               mask(rt-1) DMA-transpose (16x128 XBAR) -> maskT
               p2(rt-2): 64 accumulating fp16 matmuls outT = supp^T @ maskT
               finish(rt-2): relu(0.1 acc + 0.1 cs_half) -> PE transpose ->
               fp16 DMA out.
"""

import numpy as np

import concourse.bass as bass
import concourse.mybir as mybir
import concourse.tile as tile
from concourse import bacc
from concourse.bass_utils import run_bass_kernel_spmd
from concourse.masks import make_identity

N, DIM, HID = 8192, 256, 128
NCORES = 8
NLOC = N // NCORES          # 1024 rows per core
RT = NLOC // 128            # 8 row-tiles per core
CH = 512                    # phase-0 column chunk
NCH = N // CH               # 16
ZC = 1024                   # z psum/copy chunk
NZC = N // ZC               # 8
JT = N // 128               # 64
F32 = mybir.dt.float32
F16 = mybir.dt.float16
NEG = -1.0e30

SIGN_HI = 2048                      # ACT Sign range [0, SIGN_HI), Pool rest
COPY_ENG = ["A"] * 8                # per ZC chunk (Pool cannot read PSUM)


def build_nc():
    nc = bacc.Bacc(None, target_bir_lowering=False)

    xh_t = nc.dram_tensor("xTh", [128, 2, N], F16, kind="ExternalInput")
    xl_t = nc.dram_tensor("xTl", [128, 2, N], F16, kind="ExternalInput")
    wph_t = nc.dram_tensor("W_proj_h", [128, 2, HID], F16, kind="ExternalInput")
    wpl_t = nc.dram_tensor("W_proj_l", [128, 2, HID], F16, kind="ExternalInput")
    bp_t = nc.dram_tensor("b_proj", [128, 1], F32, kind="ExternalInput")
    wc_t = nc.dram_tensor("W_conv", [128, HID], F32, kind="ExternalInput")
    out_t = nc.dram_tensor("out", [NLOC, HID], F16, kind="ExternalOutput")

    with tile.TileContext(nc) as tc:
        with (
            tc.tile_pool(name="const", bufs=1) as cpool,
            tc.tile_pool(name="persist", bufs=1) as pers,
        ):
            ones16 = cpool.tile([128, 1], F16)
            nc.vector.memset(ones16[:], 1.0)
            two_ = cpool.tile([128, 1], F32)
            nc.vector.memset(two_[:], 2.0)
            wph_sb = cpool.tile([128, 2, HID], F16)
            nc.sync.dma_start(wph_sb[:], wph_t[:])
            wpl_sb = cpool.tile([128, 2, HID], F16)
            nc.sync.dma_start(wpl_sb[:], wpl_t[:])
            b_sb = cpool.tile([128, 1], F32)
            nc.sync.dma_start(b_sb[:], bp_t[:])
            wc_sb = cpool.tile([128, HID], F32)
            nc.sync.dma_start(wc_sb[:], wc_t[:])
            wc16 = cpool.tile([128, HID], F16)
            nc.vector.tensor_copy(wc16[:], wc_sb[:])

            hnT16 = pers.tile([128, N], F16)             # keys; queries = cols 0-1023
            supp = pers.tile([128, JT, HID], F16)        # supp rows [j%128, jt, hid]
            csb = pers.tile([128, 1], F32)               # 0.1 * cs_half bias

            # ---------------- phase 0 ----------------
            with (
                tc.tile_pool(name="xpool", bufs=3) as xpool,
                tc.tile_pool(name="hpool", bufs=1) as hpool,
                tc.tile_pool(name="p0sb", bufs=10) as p0sb,
                tc.tile_pool(name="p0norm", bufs=1) as p0n,
                tc.tile_pool(name="hppsum", bufs=3, space="PSUM") as hpp,
                tc.tile_pool(name="n2psum", bufs=3, space="PSUM") as n2p,
                tc.tile_pool(name="sppsum", bufs=2, space="PSUM") as spp,
            ):
                hT = hpool.tile([128, N], F32)
                h16 = hpool.tile([128, N], F16)
                suppT16 = hpool.tile([128, N], F16)
                su = p0n.tile([128, NCH], F32)

                for c in range(NCH):
                    csl = slice(c * CH, (c + 1) * CH)
                    if c % 2 == 0:
                        xc = xpool.tile([128, 2, ZC], F16, tag="xc")
                        nc.sync.dma_start(xc[:], xh_t[:, :, c * CH : (c + 2) * CH])
                        xcl = xpool.tile([128, 2, ZC], F16, tag="xcl")
                        nc.sync.dma_start(xcl[:], xl_t[:, :, c * CH : (c + 2) * CH])
                    xs = xc[:, :, (c % 2) * CH : ((c % 2) + 1) * CH]
                    xsl = xcl[:, :, (c % 2) * CH : ((c % 2) + 1) * CH]
                    hp = hpp.tile([128, CH], F32, tag="hp")
                    nc.tensor.matmul(hp[:], wph_sb[:, 0, :], xs[:, 0, :], start=True, stop=False)
                    nc.tensor.matmul(hp[:], wph_sb[:, 1, :], xs[:, 1, :], start=False, stop=False)
                    nc.tensor.matmul(hp[:], wpl_sb[:, 0, :], xs[:, 0, :], start=False, stop=False)
                    nc.tensor.matmul(hp[:], wpl_sb[:, 1, :], xs[:, 1, :], start=False, stop=False)
                    nc.tensor.matmul(hp[:], wph_sb[:, 0, :], xsl[:, 0, :], start=False, stop=False)
                    nc.tensor.matmul(hp[:], wph_sb[:, 1, :], xsl[:, 1, :], start=False, stop=True)
                    nc.scalar.activation(
                        hT[:, csl], hp[:], mybir.ActivationFunctionType.Relu, bias=b_sb[:]
                    )
                    nc.vector.tensor_copy(h16[:, csl], hT[:, csl])
                    sq = p0sb.tile([128, CH], F16, tag="sq")
                    nc.vector.tensor_mul(sq[:], h16[:, csl], h16[:, csl])
                    n2 = n2p.tile([1, CH], F32, tag="n2")
                    nc.tensor.matmul(n2[:], ones16[:], sq[:], start=True, stop=True)
                    s_ = p0sb.tile([1, CH], F32, tag="s_")
                    nc.scalar.activation(s_[:], n2[:], mybir.ActivationFunctionType.Sqrt)
                    rc = p0sb.tile([1, CH], F32, tag="rc")
                    nc.vector.reciprocal(rc[:], s_[:])
                    rb = p0sb.tile([128, CH], F32, tag="rb")
                    nc.gpsimd.partition_broadcast(rb[:], rc[:])
                    nc.vector.tensor_mul(hnT16[:, csl], hT[:, csl], rb[:])
                    sp = spp.tile([128, CH], F32, tag="sp")
                    nc.tensor.matmul(sp[:], wc16[:], h16[:, csl], start=True, stop=True)
                    halved = c * CH < SIGN_HI
                    nc.scalar.activation(
                        suppT16[:, csl], sp[:], mybir.ActivationFunctionType.Copy,
                        scale=0.5 if halved else 1.0,
                        accum_out=su[:, c : c + 1],
                    )
                    if c % 4 == 3:
                        k = c // 4
                        nc.sync.dma_start(
                            supp[:, k * 16 : (k + 1) * 16, :],
                            suppT16[:, k * 2048 : (k + 1) * 2048],
                            transpose=True,
                        )
                su_sum = p0n.tile([128, 1], F32)
                nc.vector.tensor_reduce(
                    su_sum[:], su[:, : SIGN_HI // CH],
                    mybir.AxisListType.X, mybir.AluOpType.add,
                )
                nc.vector.tensor_scalar_mul(csb[:], su_sum[:], 0.1)

            # ---------------- phases 1+2, software pipelined ----------------
            with (
                tc.tile_pool(name="zsb", bufs=3) as zsbp,
                tc.tile_pool(name="maskp", bufs=2) as maskp,
                tc.tile_pool(name="masktp", bufs=2) as masktp,
                tc.tile_pool(name="small", bufs=4) as sm,
                tc.tile_pool(name="outsb", bufs=3) as osb,
                tc.tile_pool(name="zpsum", bufs=3, space="PSUM") as zp,
                tc.tile_pool(name="apsum", bufs=2, space="PSUM") as ap,
            ):
                state = {}   # rt -> dict(z_sb, tau, ntau, mask, maskT)

                def stage_z(rt, pe_filler=None):
                    qsl = slice(rt * 128, (rt + 1) * 128)
                    z_sb = zsbp.tile([128, N], F32, tag="z")
                    m8p = None
                    if rt == RT - 1 or rt == 0:
                        m8p = sm.tile([128, NZC, 8], F32, tag="m8p")
                    for c in range(NZC):
                        zps = zp.tile([128, ZC], F32, tag="zp")
                        for half in range(2):
                            hsl = slice(half * CH, (half + 1) * CH)
                            jsl = slice(c * ZC + half * CH, c * ZC + (half + 1) * CH)
                            nc.tensor.matmul(
                                zps[:, hsl], hnT16[:, qsl], hnT16[:, jsl],
                                start=True, stop=True,
                            )
                        if m8p is not None:
                            nc.vector.max(m8p[:, c, :], zps[:])
                        zsl = slice(c * ZC, (c + 1) * ZC)
                        if COPY_ENG[c] == "A":
                            nc.scalar.copy(z_sb[:, zsl], zps[:])
                        elif COPY_ENG[c] == "P":
                            nc.gpsimd.tensor_copy(z_sb[:, zsl], zps[:])
                        else:
                            nc.vector.tensor_copy(z_sb[:, zsl], zps[:])
                        if pe_filler is not None and c >= NZC // 2:
                            for _ in range(JT // (NZC // 2)):
                                next(pe_filler, None)
                    state[rt] = {"z": z_sb, "m8p": m8p}

                def stage_tau(rt):
                    st = state[rt]
                    z_sb = st["z"]
                    if st["m8p"] is not None:
                        m8f = st["m8p"][:].rearrange("p a b -> p (a b)")
                        nm8 = NZC * 8
                    else:
                        m8 = sm.tile([128, 4, 8], F32, tag="m8")
                        for k in range(4):
                            nc.vector.max(m8[:, k, :], z_sb[:, k * 2048 : (k + 1) * 2048])
                        m8f = m8[:].rearrange("p a b -> p (a b)")
                        nm8 = 32
                    t8a = sm.tile([128, 8], F32, tag="t8a")
                    nc.vector.max(t8a[:], m8f)
                    m8z = sm.tile([128, 64], F32, tag="m8z")
                    nc.vector.match_replace(m8z[:, :nm8], t8a[:], m8f, NEG)
                    t8b = sm.tile([128, 8], F32, tag="t8b")
                    nc.vector.max(t8b[:], m8z[:, :nm8])
                    tsum = sm.tile([128, 1], F32, tag="tsum")
                    nc.vector.tensor_add(tsum[:], t8b[:, 1:2], t8b[:, 2:3])
                    tau = sm.tile([128, 1], F32, tag="tau")
                    nc.vector.tensor_scalar_mul(tau[:], tsum[:], 0.5)
                    ntau = sm.tile([128, 1], F32, tag="ntau")
                    nc.vector.tensor_scalar_mul(ntau[:], tsum[:], -0.5)
                    st["tau"], st["ntau"] = tau, ntau

                def stage_cmp(rt):
                    st = state[rt]
                    z_sb, tau, ntau = st["z"], st["tau"], st["ntau"]
                    tail = rt >= RT - 2
                    st["tail"] = tail
                    mask = maskp.tile([128, N], F16, tag="mask")
                    maskT = masktp.tile([128, JT, 128], F16, tag="maskT")
                    for k in range(4):
                        ksl = slice(k * 2048, (k + 1) * 2048)
                        if (k + 1) * 2048 <= SIGN_HI:
                            if tail:
                                # {0,2} mask on the pre-halved range: no bias needed
                                nc.vector.tensor_scalar(
                                    mask[:, ksl], z_sb[:, ksl], tau[:], two_[:],
                                    op0=mybir.AluOpType.is_ge,
                                    op1=mybir.AluOpType.mult,
                                )
                            else:
                                nc.scalar.activation(
                                    mask[:, ksl], z_sb[:, ksl],
                                    mybir.ActivationFunctionType.Sign, bias=ntau[:],
                                )
                        elif tail and k == 1:
                            nc.vector.tensor_scalar(
                                mask[:, ksl], z_sb[:, ksl], tau[:], None,
                                op0=mybir.AluOpType.is_ge,
                            )
                        else:
                            nc.gpsimd.tensor_scalar(
                                mask[:, ksl], z_sb[:, ksl], tau[:], None,
                                op0=mybir.AluOpType.is_ge,
                            )
                        nc.sync.dma_start(
                            maskT[:, k * 16 : (k + 1) * 16, :], mask[:, ksl],
                            transpose=True,
                        )
                    st["maskT"] = maskT

                def p2_mms(rt):
                    st = state[rt]
                    maskT = st["maskT"]
                    acc = ap.tile([128, 128], F32, tag="acc")
                    st["acc"] = acc
                    for jt in range(JT):
                        nc.tensor.matmul(
                            acc[:], supp[:, jt, :], maskT[:, jt, :],
                            start=(jt == 0), stop=(jt == JT - 1),
                        )
                        yield

                def stage_p2_finish(rt):
                    st = state[rt]
                    acc = st["acc"]
                    oT = osb.tile([128, 128], F16, tag="oT")
                    nc.scalar.activation(
                        oT[:], acc[:], mybir.ActivationFunctionType.Relu,
                        bias=0.0 if st["tail"] else csb[:], scale=0.1,
                    )
                    o_sb = osb.tile([128, 128], F16, tag="osb")
                    nc.scalar.dma_start(o_sb[:], oT[:], transpose=True)
                    nc.scalar.dma_start(out_t[rt * 128 : (rt + 1) * 128, :], o_sb[:])
                    del state[rt]

                for rt in range(RT):
                    filler = p2_mms(rt - 2) if rt >= 2 else None
                    stage_z(rt, filler)
                    if filler is not None:
                        for _ in filler:
                            pass
                        stage_p2_finish(rt - 2)
                    stage_tau(rt)
                    if rt >= 1:
                        stage_cmp(rt - 1)
                stage_cmp(RT - 1)
                for rt in (RT - 2, RT - 1):
                    for _ in p2_mms(rt):
                        pass
                    stage_p2_finish(rt)

    nc.compile()
    return nc


_NC_CACHE = {}


def kernel(x, W_proj, b_proj, W_conv):
    if "nc" not in _NC_CACHE:
        _NC_CACHE["nc"] = build_nc()
    nc = _NC_CACHE["nc"]
    x = np.ascontiguousarray(x, dtype=np.float32)
    W_proj = np.ascontiguousarray(W_proj, dtype=np.float32)
    b_proj = np.ascontiguousarray(b_proj, dtype=np.float32)
    W_conv = np.ascontiguousarray(W_conv, dtype=np.float32)

    wp3 = W_proj.reshape(2, 128, HID).transpose(1, 0, 2)
    wph = wp3.astype(np.float16)
    wpl = (wp3 - wph.astype(np.float32)).astype(np.float16)
    b_dev = np.ascontiguousarray(b_proj.reshape(HID, 1))
    in_maps = []
    for c in range(NCORES):
        x_rot = np.concatenate([x[c * NLOC :], x[: c * NLOC]], axis=0)
        xT = np.ascontiguousarray(x_rot.T)            # [256, 8192]
        xT3 = xT.reshape(2, 128, N).transpose(1, 0, 2)
        xh = np.ascontiguousarray(xT3.astype(np.float16))
        xl = np.ascontiguousarray((xT3 - xh.astype(np.float32)).astype(np.float16))
        in_maps.append(
            {"xTh": xh, "xTl": xl, "W_proj_h": np.ascontiguousarray(wph),
             "W_proj_l": np.ascontiguousarray(wpl), "b_proj": b_dev, "W_conv": W_conv}
        )
    res = run_bass_kernel_spmd(nc, in_maps, core_ids=list(range(NCORES)))
    return np.concatenate(
        [res.results[c]["out"].astype(np.float32) for c in range(NCORES)], axis=0
    )
